# revision 24
# baseline (speedup 1.0000x reference)
"""Bidirectional chamfer distance (nn_DisplacementLoss) on 8 trn2 NeuronCores.

Sharding: 8 cores = 4 batches x 2 directions. Core c handles batch c%4,
direction c//4 (0: pred->gt, 1: gt->pred). Each core computes the row-mins
of its 5000x5000 squared-distance matrix via a K=5 augmented fp32r matmul
(d2 = |x|^2 + |y|^2 - 2<x,y> folded into one contraction; fp32r runs the PE
at 1 cycle/column vs 4 for fp32) tiled 128x512 into PSUM. The tiny host
gather averages the per-core row-min vectors into the scalar loss.

Min-reduction pipeline per 128-row n-tile (m = 5120 columns), mode
"tail_dve": two 1024-wide PSUM groups are paired against two 1024-wide
groups that ScalarE copies PSUM->SBUF, via a registered custom DVE op
TT_MIN_REDUCE_ANT (out = min(in0,in1); accum_out = running row min, with
in0 on the PSUM read port and in1 on an SBUF port -> DVE consumes 2
elements/cycle); the remaining 1024-wide group is consumed directly from
PSUM against a BIG constant tile. The native TENSOR_TENSOR_REDUCE ISA op
hard-crashes this runtime (NRT_EXEC_UNIT_UNRECOVERABLE), hence the custom
op. Measured ~170 us/core on hardware (wall-clock slope over an on-device
repeat loop), vs ~240 us for a plain reduce-from-PSUM pipeline.
"""

import numpy as np

B, N, D = 4, 5000, 3
NP = 5120  # padded pred points: 40 tiles x 128 partitions
MP = 5120  # padded gt points: 10 chunks x 512
NT = NP // 128
K = 5  # augmented contraction: [-2x0,-2x1,-2x2, x2, 1] . [y0,y1,y2, 1, y2sum]
BIG = 1.0e30

_compiled = None
_ttmin_op = None


def _register_tt_min_reduce():
    """Custom DVE op: out = min(in0,in1); accum_out = min(s0, min_k out[k]).
    2-input 1x DVE op (rd0+rd1) - consumes two streams per cycle while
    producing the running row-min in accum_out."""
    global _ttmin_op
    if _ttmin_op is not None:
        return _ttmin_op
    import concourse.dve_ops as dops
    from concourse.dve_spec import Spec, Src0, Src1, C0, minn, _has_src1, lower
    from concourse.dve_uop import DveOpSpec

    for op in dops.OPS:
        if op.name == "TT_MIN_REDUCE_ANT":
            _ttmin_op = op
            return op

    def _ref(in0, in1, c0, c1, c2):
        b = np.minimum(in0.astype(np.float32), in1.astype(np.float32))
        acc = np.minimum(
            np.asarray(c0, dtype=np.float32),
            b.reshape(b.shape[0], -1).min(axis=-1, keepdims=True),
        ).astype(np.float32)
        return b, acc

    spec = Spec(body=minn(Src0, Src1), accum=minn, accum_init=C0, reference=_ref)
    op = dops.DveOp("TT_MIN_REDUCE_ANT", spec, subdim=False, uops_sha={})
    dops.OPS.append(op)
    dops.CUSTOM_DVE_SPECS[op.name] = spec
    row = dops._CUSTOM_DVE_ROW_BASE + len(dops.OPS) - 1
    assert row < 0x20
    dops._SUB_OPCODE_FOR_NAME[op.name] = row
    for ver in ("v3", "v4"):
        tmp = DveOpSpec(
            name=op.name, opcode=row, uops=lower(spec, ver=ver),
            rd1_en=_has_src1(spec),
        )
        op.uops_sha[ver] = tmp.sha(ver)
    _ttmin_op = op
    return op


def _build_program(repeat=None, mode="tail_dve", big_bufs=False):
    import contextlib

    import concourse.bacc as bacc
    import concourse.tile as tile
    import concourse.mybir as mybir

    f32 = mybir.dt.float32
    f32r = mybir.dt.float32r
    ttmin = _register_tt_min_reduce()
    nc = bacc.Bacc(debug=False, num_devices=8)
    a_dram = nc.dram_tensor("a_aug", [K, NP], f32r, kind="ExternalInput").ap()
    b_dram = nc.dram_tensor("b_aug", [K, MP], f32r, kind="ExternalInput").ap()
    out_dram = nc.dram_tensor("minvals", [128, NT], f32, kind="ExternalOutput").ap()

    # Per n-tile the 5120-wide m-row is processed as 3 (dve, act) group
    # pairs: the dve group stays in PSUM (TTR in0), the act group is copied
    # to SBUF by ScalarE (TTR in1). Group widths 1024,1024,512.
    pairs = [(0, 1024), (2048, 1024), (4096, 512)]  # (dve group offset, width)

    merged = mode in ("merged", "paired25")
    pa_bufs = 1 if merged else 2
    stage_bufs = 4 if big_bufs else 3
    scratch_bufs = 3 if big_bufs else 2
    acc_bufs = 3 if big_bufs else 2
    with tile.TileContext(nc) as tc:
        with (
            tc.tile_pool(name="const", bufs=1) as const_pool,
            tc.tile_pool(name="acc", bufs=acc_bufs) as acc_pool,
            tc.tile_pool(name="stage", bufs=stage_bufs) as stage_pool,
            tc.tile_pool(name="scratch", bufs=scratch_bufs) as scratch_pool,
            tc.tile_pool(name="psum_d", bufs=2, space="PSUM") as psum_d_pool,
            tc.tile_pool(name="psum_a", bufs=pa_bufs, space="PSUM") as psum_a_pool,
        ):
            a_sb = const_pool.tile([K, NP], f32r)
            nc.sync.dma_start(a_sb[:], a_dram[:])
            b_sb = const_pool.tile([K, MP], f32r)
            # split the load so the first m-groups' matmuls start sooner
            nc.sync.dma_start(b_sb[:, :2048], b_dram[:, :2048])
            nc.sync.dma_start(b_sb[:, 2048:], b_dram[:, 2048:])
            out_sb = const_pool.tile([128, NT], f32)
            bigs = const_pool.tile([128, 1024], f32)
            nc.vector.memset(bigs[:], BIG)

            # Optional benchmark mode: repeat the (idempotent) compute body
            # R times inside a dynamic loop so per-iteration HW time can be
            # measured from the wall-clock slope between two R values.
            rep_ctx = (
                tc.For_i(0, repeat, 1) if repeat is not None else contextlib.nullcontext()
            )
            with rep_ctx:
                _emit_body(
                    nc, tile, mybir, ttmin, pairs,
                    a_sb, b_sb, out_sb, bigs,
                    acc_pool, stage_pool, scratch_pool, psum_d_pool, psum_a_pool,
                    mode,
                )
            nc.sync.dma_start(out_dram[:], out_sb[:])

    nc.compile()
    return nc


def _emit_body(nc, tile, mybir, ttmin, pairs, a_sb, b_sb, out_sb, bigs,
               acc_pool, stage_pool, scratch_pool, psum_d_pool, psum_a_pool,
               mode):
    f32 = mybir.dt.float32

    def mm(ptile, lhsT, m0, w):
        o = 0
        while o < w:
            c = min(512, w - o)
            nc.tensor.matmul(
                ptile[:, o : o + c],
                lhsT,
                b_sb[:, m0 + o : m0 + o + c],
                start=True,
                stop=True,
            )
            o += c

    def ttr(scr_w, in0, in1, s0, acc_out):
        scr = scratch_pool.tile([128, 1024], f32, name="scr")
        nc.vector._custom_dve(
            ttmin, out=scr[:, :scr_w], in0=in0, in1=in1, s0=s0, accum_out=acc_out
        )

    if mode in ("merged", "paired25"):
        for nt in range(NT):
            lhsT = a_sb[:, nt * 128 : (nt + 1) * 128]
            out_col = out_sb[:, nt : nt + 1]
            # act-side: one contiguous 2048 group, single ScalarE copy
            pa = psum_a_pool.tile([128, 2048], f32, name="pabig")
            mm(pa, lhsT, 2048, 2048)
            stagedA = stage_pool.tile([128, 2048], f32, name="stA")
            nc.scalar.copy(stagedA[:], pa[:])
            # dve-side groups
            pd0 = psum_d_pool.tile([128, 1024], f32, name="pg0")
            mm(pd0, lhsT, 0, 1024)
            pd1 = psum_d_pool.tile([128, 1024], f32, name="pg0")
            mm(pd1, lhsT, 1024, 1024)
            acc1 = acc_pool.tile([128, 1], f32, name="acc")
            ttr(1024, pd0[:], stagedA[:, :1024], BIG, acc1)
            acc2 = acc_pool.tile([128, 1], f32, name="acc")
            ttr(1024, pd1[:], stagedA[:, 1024:2048], acc1, acc2)
            if mode == "merged":
                pdt = psum_d_pool.tile([128, 1024], f32, name="pg0")
                mm(pdt, lhsT, 4096, 1024)
                ttr(1024, pdt[:], bigs[:, :1024], acc2, out_col)
            else:
                pdt = psum_d_pool.tile([128, 1024], f32, name="pg0")
                mm(pdt, lhsT, 4096, 512)
                pat = psum_d_pool.tile([128, 1024], f32, name="pg0")
                mm(pat, lhsT, 4608, 512)
                staged2 = stage_pool.tile([128, 512], f32, name="st2")
                nc.scalar.copy(staged2[:], pat[:, :512])
                ttr(512, pdt[:, :512], staged2[:], acc2, out_col)
        return

    for nt in range(NT):
        lhsT = a_sb[:, nt * 128 : (nt + 1) * 128]
        prev_acc = None

        nochain = mode.endswith("_nc")
        base = mode[:-3] if nochain else mode
        if base == "noact":
            # every 1024-group consumed directly by DVE (in1 = BIG consts)
            groups = [(0, 1024), (1024, 1024), (2048, 1024), (3072, 1024), (4096, 1024)]
        elif base in ("tail_dve", "tail"):
            # tail covers real columns 4096..4999 only (padded cols skipped)
            groups = pairs[:2] + [("tail", N - 4096)]
        elif base == "tail_first":
            # ACT-independent direct group first: DVE never waits for the
            # staged copy at tile start
            groups = [("tail", 1024)] + pairs[:2]
        else:
            groups = pairs
        rm = acc_pool.tile([128, 4], f32, name="rm") if nochain else None

        n_groups = len(groups)
        for pi, (d0, w) in enumerate(groups):
            last = pi == n_groups - 1
            if nochain:
                acc_out = rm[:, pi : pi + 1]
            elif last:
                acc_out = out_sb[:, nt : nt + 1]
            else:
                acc_out = acc_pool.tile([128, 1], f32)
            scratch = scratch_pool.tile([128, 1024], f32)

            if base == "noact":
                pool = psum_d_pool if pi % 2 == 0 else psum_a_pool
                pd = pool.tile([128, 1024], f32, name=f"pg{pi % 2}")
                mm(pd, lhsT, d0, w)
                in1 = bigs[:, :w]
            elif d0 == "tail":
                pd = psum_d_pool.tile([128, 1024], f32, name="pg0")
                mm(pd, lhsT, 4096, w)
                in1 = bigs[:, :w]
            else:
                a0 = d0 + w  # act group sits right after the dve group
                pd = psum_d_pool.tile([128, 1024], f32, name="pg0")
                pa = psum_a_pool.tile([128, 1024], f32, name="pg1")
                for i in range(w // 512):
                    nc.tensor.matmul(
                        pd[:, 512 * i : 512 * (i + 1)], lhsT,
                        b_sb[:, d0 + 512 * i : d0 + 512 * (i + 1)],
                        start=True, stop=True,
                    )
                    nc.tensor.matmul(
                        pa[:, 512 * i : 512 * (i + 1)], lhsT,
                        b_sb[:, a0 + 512 * i : a0 + 512 * (i + 1)],
                        start=True, stop=True,
                    )
                staged = stage_pool.tile([128, 1024], f32)
                nc.scalar.copy(staged[:, :w], pa[:, :w])
                in1 = staged[:, :w]

            nc.vector._custom_dve(
                ttmin,
                out=scratch[:, :w],
                in0=pd[:, :w],
                in1=in1,
                s0=BIG if (nochain or prev_acc is None) else prev_acc,
                accum_out=acc_out,
            )
            prev_acc = acc_out
        if nochain:
            nc.vector.tensor_reduce(
                out=out_sb[:, nt : nt + 1],
                in_=rm[:, :n_groups],
                axis=mybir.AxisListType.X,
                op=mybir.AluOpType.min,
            )


def _get_program():
    global _compiled
    if _compiled is None:
        _compiled = _build_program()
    return _compiled


def _make_core_inputs(x, y):
    """x: query points [N,3] f32, y: database points [N,3] f32.
    Returns (a_aug [K,NP], b_aug [K,MP]) so that (a_aug.T @ b_aug)[n,m] = d2."""
    a = np.zeros((K, NP), dtype=np.float32)
    a[0:3, :N] = -2.0 * x.T
    a[3, :N] = (x * x).sum(axis=1)
    a[4, :N] = 1.0
    b = np.zeros((K, MP), dtype=np.float32)
    b[0:3, :N] = y.T
    b[3, :N] = 1.0
    b[4, :N] = (y * y).sum(axis=1)
    b[4, N:] = BIG  # padded columns never win the min
    return a, b


def _run(pred_samples, gt_samples, trace=False):
    from concourse.bass_utils import run_bass_kernel_spmd

    nc = _get_program()
    pred = np.asarray(pred_samples, dtype=np.float32)
    gt = np.asarray(gt_samples, dtype=np.float32)
    in_maps = []
    for c in range(8):
        bidx = c % 4
        if c < 4:
            a, bb = _make_core_inputs(pred[bidx], gt[bidx])
        else:
            a, bb = _make_core_inputs(gt[bidx], pred[bidx])
        in_maps.append({"a_aug": a, "b_aug": bb})
    res = run_bass_kernel_spmd(nc, in_maps, list(range(8)), trace=trace)
    return res


def _gather(res):
    total = 0.0
    for c in range(8):
        mv = res.results[c]["minvals"]  # [128, NT]
        mins = mv.transpose(1, 0).reshape(-1)[:N].astype(np.float64)
        mins = np.maximum(mins, 0.0)
        total += mins.mean()
    return np.float32(total / 4.0)


def kernel(pred_samples, gt_samples):
    res = _run(pred_samples, gt_samples)
    return _gather(res)


# revision 41
# speedup vs baseline: 1.1086x; 1.1086x over previous
"""Bidirectional chamfer distance (nn_DisplacementLoss) on 8 trn2 NeuronCores.

Sharding: 8 cores = 4 batches x 2 directions. Core c handles batch c%4,
direction c//4 (0: pred->gt, 1: gt->pred). Each core computes the row-mins
of its 5000x5000 squared-distance matrix via a K=5 augmented fp32r matmul
(d2 = |x|^2 + |y|^2 - 2<x,y> folded into one contraction; fp32r runs the PE
at 1 cycle/column vs 4 for fp32) tiled 128x512 into PSUM. The tiny host
gather averages the per-core row-min vectors into the scalar loss.

Min-reduction (shipping mode "decomp2"): min(a,b) = (a+b)/2 - |a-b|/2,
where s = (a+b)/2 and d = (a-b)/2 are linear in the gt points, so the PE
computes them directly as matmuls against sum/diff-augmented pair-columns
(2500 pairs per 128-row tile). ScalarE computes |d| PSUM->SBUF (Abs), and
the registered custom DVE op TT_SUB_MIN_ANT (out = in0 - in1; accum_out =
running row min, in0 on the PSUM read port, in1 on an SBUF port) consumes
the (s, |d|) streams at 2 elements/cycle with no reduced-rate tail.
Groups (452,1024,1024) keep both PSUM pools double-buffered in exactly 8
banks. The native TENSOR_TENSOR_REDUCE ISA op hard-crashes this runtime
(NRT_EXEC_UNIT_UNRECOVERABLE), hence the custom ops. Measured ~168-188
us/core on hardware (wall-clock slope over an on-device repeat loop, load
dependent), vs ~275 us for a plain reduce-from-PSUM pipeline; several
alternative pipelines (copy-pairing "tail_dve" etc.) remain selectable
via _build_program(mode=...) and measure within a few percent.
"""

import numpy as np

B, N, D = 4, 5000, 3
NP = 5120  # padded pred points: 40 tiles x 128 partitions
MP = 5120  # padded gt points: 10 chunks x 512
NT = NP // 128
K = 5  # augmented contraction: [-2x0,-2x1,-2x2, x2, 1] . [y0,y1,y2, 1, y2sum]
BIG = 1.0e30

_compiled = None
_ttmin_op = None
_ttsub_op = None
PAIRS_M = 2560  # decomp mode: number of (y_2j, y_2j+1) pair-columns (2500 real)
REAL_PAIRS = 2500


def _register_tt_min_reduce():
    """Custom DVE op: out = min(in0,in1); accum_out = min(s0, min_k out[k]).
    2-input 1x DVE op (rd0+rd1) - consumes two streams per cycle while
    producing the running row-min in accum_out."""
    global _ttmin_op
    if _ttmin_op is not None:
        return _ttmin_op
    import concourse.dve_ops as dops
    from concourse.dve_spec import Spec, Src0, Src1, C0, minn, _has_src1, lower
    from concourse.dve_uop import DveOpSpec

    for op in dops.OPS:
        if op.name == "TT_MIN_REDUCE_ANT":
            _ttmin_op = op
            return op

    def _ref(in0, in1, c0, c1, c2):
        b = np.minimum(in0.astype(np.float32), in1.astype(np.float32))
        acc = np.minimum(
            np.asarray(c0, dtype=np.float32),
            b.reshape(b.shape[0], -1).min(axis=-1, keepdims=True),
        ).astype(np.float32)
        return b, acc

    def _reg(name, spec):
        op = dops.DveOp(name, spec, subdim=False, uops_sha={})
        dops.OPS.append(op)
        dops.CUSTOM_DVE_SPECS[op.name] = spec
        row = dops._CUSTOM_DVE_ROW_BASE + len(dops.OPS) - 1
        assert row < 0x20
        dops._SUB_OPCODE_FOR_NAME[op.name] = row
        for ver in ("v3", "v4"):
            tmp = DveOpSpec(
                name=op.name, opcode=row, uops=lower(spec, ver=ver),
                rd1_en=_has_src1(spec),
            )
            op.uops_sha[ver] = tmp.sha(ver)
        return op

    _ttmin_op = _reg(
        "TT_MIN_REDUCE_ANT",
        Spec(body=minn(Src0, Src1), accum=minn, accum_init=C0, reference=_ref),
    )

    # out = in0 - in1; accum_out = min(s0, min_k out[k]).
    # With in0 = (a+b)/2 and in1 = |a-b|/2, out is min(a,b) pairwise.
    def _ref_sub(in0, in1, c0, c1, c2):
        b = (in0.astype(np.float32) - in1.astype(np.float32)).astype(np.float32)
        acc = np.minimum(
            np.asarray(c0, dtype=np.float32),
            b.reshape(b.shape[0], -1).min(axis=-1, keepdims=True),
        ).astype(np.float32)
        return b, acc

    global _ttsub_op
    _ttsub_op = _reg(
        "TT_SUB_MIN_ANT",
        Spec(body=Src0 - Src1, accum=minn, accum_init=C0, reference=_ref_sub),
    )
    return _ttmin_op


SHIP_MODE = "decomp2"       # kernel mode used by kernel()
SHIP_INPUT_MODE = "decomp"  # matching host-side input layout


def _build_program(repeat=None, mode=SHIP_MODE, big_bufs=False):
    import contextlib

    import concourse.bacc as bacc
    import concourse.tile as tile
    import concourse.mybir as mybir

    f32 = mybir.dt.float32
    f32r = mybir.dt.float32r
    ttmin = _register_tt_min_reduce()
    nc = bacc.Bacc(debug=False, num_devices=8)
    a_dram = nc.dram_tensor("a_aug", [K, NP], f32r, kind="ExternalInput").ap()
    b_dram = nc.dram_tensor("b_aug", [K, MP], f32r, kind="ExternalInput").ap()
    out_dram = nc.dram_tensor("minvals", [128, NT], f32, kind="ExternalOutput").ap()

    # Per n-tile the 5120-wide m-row is processed as 3 (dve, act) group
    # pairs: the dve group stays in PSUM (TTR in0), the act group is copied
    # to SBUF by ScalarE (TTR in1). Group widths 1024,1024,512.
    pairs = [(0, 1024), (2048, 1024), (4096, 512)]  # (dve group offset, width)

    merged = mode in ("merged", "paired25")
    pa_bufs = 1 if merged else 2
    stage_bufs = 4 if big_bufs else 3
    scratch_bufs = 3 if big_bufs else 2
    acc_bufs = 3 if big_bufs else 2
    with tile.TileContext(nc) as tc, contextlib.ExitStack() as es:
        const_pool = es.enter_context(tc.tile_pool(name="const", bufs=1))
        acc_pool = es.enter_context(tc.tile_pool(name="acc", bufs=acc_bufs))
        stage_pool = es.enter_context(tc.tile_pool(name="stage", bufs=stage_bufs))
        scratch_pool = es.enter_context(tc.tile_pool(name="scratch", bufs=scratch_bufs))
        # PSUM bank budget is 8: default modes use 4+4 (two [128,1024]x2
        # pools); decomp mode uses 4+1+3 (shared [1024]x2 + [512]x1 + [1536]x1)
        psum_d_pool = es.enter_context(tc.tile_pool(name="psum_d", bufs=2, space="PSUM"))
        if mode == "decomp":
            ps_small = es.enter_context(tc.tile_pool(name="ps_small", bufs=1, space="PSUM"))
            ps_dbig = es.enter_context(tc.tile_pool(name="ps_dbig", bufs=1, space="PSUM"))
            psum_a_pool = None
        elif mode in ("decomp2", "decomp2_nc"):
            # all-double-buffered variant: d-groups live in their own
            # [128,1024]x2 pool, same shape as the s pool
            psum_a_pool = es.enter_context(tc.tile_pool(name="psum_a", bufs=2, space="PSUM"))
            ps_small = ps_dbig = None
        else:
            psum_a_pool = es.enter_context(
                tc.tile_pool(name="psum_a", bufs=pa_bufs, space="PSUM")
            )
            ps_small = ps_dbig = None
        if True:
            a_sb = const_pool.tile([K, NP], f32r)
            nc.sync.dma_start(a_sb[:], a_dram[:])
            b_sb = const_pool.tile([K, MP], f32r)
            # split the load so the first m-groups' matmuls start sooner
            nc.sync.dma_start(b_sb[:, :2048], b_dram[:, :2048])
            nc.sync.dma_start(b_sb[:, 2048:], b_dram[:, 2048:])
            out_sb = const_pool.tile([128, NT], f32)
            bigs = const_pool.tile([128, 1024], f32)
            nc.vector.memset(bigs[:], BIG)

            # Optional benchmark mode: repeat the (idempotent) compute body
            # R times inside a dynamic loop so per-iteration HW time can be
            # measured from the wall-clock slope between two R values.
            rep_ctx = (
                tc.For_i(0, repeat, 1) if repeat is not None else contextlib.nullcontext()
            )
            with rep_ctx:
                if mode == "decomp":
                    _emit_decomp(
                        nc, mybir, a_sb, b_sb, out_sb,
                        acc_pool, stage_pool, scratch_pool,
                        psum_d_pool, ps_small, ps_dbig,
                    )
                elif mode in ("decomp2", "decomp2_nc"):
                    _emit_decomp2(
                        nc, mybir, a_sb, b_sb, out_sb,
                        acc_pool, stage_pool, scratch_pool,
                        psum_d_pool, psum_a_pool,
                        nochain=(mode == "decomp2_nc"),
                    )
                else:
                    _emit_body(
                        nc, tile, mybir, ttmin, pairs,
                        a_sb, b_sb, out_sb, bigs,
                        acc_pool, stage_pool, scratch_pool, psum_d_pool, psum_a_pool,
                        mode,
                    )
            nc.sync.dma_start(out_dram[:], out_sb[:])

    nc.compile()
    return nc


def _emit_decomp(nc, mybir, a_sb, b_sb, out_sb,
                 acc_pool, stage_pool, scratch_pool,
                 psum_shared, ps_small, ps_dbig):
    """min(a,b) = (a+b)/2 - |a-b|/2 with s=(a+b)/2, d=(a-b)/2 computed as
    matmuls against sum/diff-augmented gt pair-columns (both linear in the
    rhs). ScalarE computes |d| PSUM->SBUF; DVE pairs (s, |d|) at 2 elem/cyc
    via TT_SUB_MIN_ANT. All 2500 pair-values per tile are consumed at the
    paired rate - no 1x-rate tail."""
    f32 = mybir.dt.float32
    ttsub = _ttsub_op
    DOFF = PAIRS_M  # diff region offset in b_sb

    def mm(ptile, lhsT, m0, w):
        o = 0
        while o < w:
            c = min(512, w - o)
            nc.tensor.matmul(
                ptile[:, o : o + c], lhsT,
                b_sb[:, m0 + o : m0 + o + c],
                start=True, stop=True,
            )
            o += c

    ABS = mybir.ActivationFunctionType.Abs
    for nt in range(NT):
        lhsT = a_sb[:, nt * 128 : (nt + 1) * 128]
        out_col = out_sb[:, nt : nt + 1]
        # diff side first so ScalarE's |d| is staged before DVE needs it
        d0 = psum_shared.tile([128, 1024], f32, name="pg0")
        mm(d0, lhsT, DOFF, 1024)
        dbig = ps_dbig.tile([128, 1476], f32, name="dbig")
        mm(dbig, lhsT, DOFF + 1024, REAL_PAIRS - 1024)
        staged = stage_pool.tile([128, REAL_PAIRS], f32, name="stD")
        nc.scalar.activation(staged[:, :1024], d0[:], ABS)
        nc.scalar.activation(staged[:, 1024:REAL_PAIRS], dbig[:, : REAL_PAIRS - 1024], ABS)
        # sum side
        s0 = psum_shared.tile([128, 1024], f32, name="pg0")
        mm(s0, lhsT, 0, 1024)
        s1 = psum_shared.tile([128, 1024], f32, name="pg0")
        mm(s1, lhsT, 1024, 1024)
        s2 = ps_small.tile([128, 512], f32, name="ssml")
        mm(s2, lhsT, 2048, REAL_PAIRS - 2048)
        w2 = REAL_PAIRS - 2048
        acc1 = acc_pool.tile([128, 1], f32, name="acc")
        scr = scratch_pool.tile([128, 1024], f32, name="scr")
        nc.vector._custom_dve(
            ttsub, out=scr[:, :1024], in0=s0[:], in1=staged[:, :1024],
            s0=BIG, accum_out=acc1,
        )
        acc2 = acc_pool.tile([128, 1], f32, name="acc")
        scr = scratch_pool.tile([128, 1024], f32, name="scr")
        nc.vector._custom_dve(
            ttsub, out=scr[:, :1024], in0=s1[:], in1=staged[:, 1024:2048],
            s0=acc1, accum_out=acc2,
        )
        scr = scratch_pool.tile([128, 1024], f32, name="scr")
        nc.vector._custom_dve(
            ttsub, out=scr[:, :w2], in0=s2[:, :w2], in1=staged[:, 2048:REAL_PAIRS],
            s0=acc2, accum_out=out_col,
        )


def _emit_decomp2(nc, mybir, a_sb, b_sb, out_sb,
                  acc_pool, stage_pool, scratch_pool, s_pool, d_pool,
                  nochain=False):
    """Decomposed-min variant with every PSUM pool double-buffered:
    s-groups and d-groups both (452,1024,1024) in [128,1024]x2 pools.
    The small group goes first so its |d| is staged earliest."""
    f32 = mybir.dt.float32
    ttsub = _ttsub_op
    DOFF = PAIRS_M
    groups = [(0, REAL_PAIRS - 2048), (REAL_PAIRS - 2048, 1024), (REAL_PAIRS - 1024, 1024)]

    def mm(ptile, lhsT, m0, w):
        o = 0
        while o < w:
            c = min(512, w - o)
            nc.tensor.matmul(
                ptile[:, o : o + c], lhsT,
                b_sb[:, m0 + o : m0 + o + c],
                start=True, stop=True,
            )
            o += c

    ABS = mybir.ActivationFunctionType.Abs
    for nt in range(NT):
        lhsT = a_sb[:, nt * 128 : (nt + 1) * 128]
        staged = stage_pool.tile([128, REAL_PAIRS], f32, name="stD")
        for (g0, w) in groups:
            dt_ = d_pool.tile([128, 1024], f32, name="pga")
            mm(dt_, lhsT, DOFF + g0, w)
            nc.scalar.activation(staged[:, g0 : g0 + w], dt_[:, :w], ABS)
        rm = acc_pool.tile([128, 4], f32, name="rm") if nochain else None
        prev = None
        for gi, (g0, w) in enumerate(groups):
            st = s_pool.tile([128, 1024], f32, name="pg0")
            mm(st, lhsT, g0, w)
            scr = scratch_pool.tile([128, 1024], f32, name="scr")
            if nochain:
                acc_out = rm[:, gi : gi + 1]
            elif gi == len(groups) - 1:
                acc_out = out_sb[:, nt : nt + 1]
            else:
                acc_out = acc_pool.tile([128, 1], f32, name="acc")
            nc.vector._custom_dve(
                ttsub, out=scr[:, :w], in0=st[:, :w], in1=staged[:, g0 : g0 + w],
                s0=BIG if (nochain or prev is None) else prev, accum_out=acc_out,
            )
            prev = acc_out
        if nochain:
            nc.vector.tensor_reduce(
                out=out_sb[:, nt : nt + 1],
                in_=rm[:, : len(groups)],
                axis=mybir.AxisListType.X,
                op=mybir.AluOpType.min,
            )


def _emit_body(nc, tile, mybir, ttmin, pairs, a_sb, b_sb, out_sb, bigs,
               acc_pool, stage_pool, scratch_pool, psum_d_pool, psum_a_pool,
               mode):
    f32 = mybir.dt.float32

    def mm(ptile, lhsT, m0, w):
        o = 0
        while o < w:
            c = min(512, w - o)
            nc.tensor.matmul(
                ptile[:, o : o + c],
                lhsT,
                b_sb[:, m0 + o : m0 + o + c],
                start=True,
                stop=True,
            )
            o += c

    def ttr(scr_w, in0, in1, s0, acc_out):
        scr = scratch_pool.tile([128, 1024], f32, name="scr")
        nc.vector._custom_dve(
            ttmin, out=scr[:, :scr_w], in0=in0, in1=in1, s0=s0, accum_out=acc_out
        )

    if mode in ("merged", "paired25"):
        for nt in range(NT):
            lhsT = a_sb[:, nt * 128 : (nt + 1) * 128]
            out_col = out_sb[:, nt : nt + 1]
            # act-side: one contiguous 2048 group, single ScalarE copy
            pa = psum_a_pool.tile([128, 2048], f32, name="pabig")
            mm(pa, lhsT, 2048, 2048)
            stagedA = stage_pool.tile([128, 2048], f32, name="stA")
            nc.scalar.copy(stagedA[:], pa[:])
            # dve-side groups
            pd0 = psum_d_pool.tile([128, 1024], f32, name="pg0")
            mm(pd0, lhsT, 0, 1024)
            pd1 = psum_d_pool.tile([128, 1024], f32, name="pg0")
            mm(pd1, lhsT, 1024, 1024)
            acc1 = acc_pool.tile([128, 1], f32, name="acc")
            ttr(1024, pd0[:], stagedA[:, :1024], BIG, acc1)
            acc2 = acc_pool.tile([128, 1], f32, name="acc")
            ttr(1024, pd1[:], stagedA[:, 1024:2048], acc1, acc2)
            if mode == "merged":
                pdt = psum_d_pool.tile([128, 1024], f32, name="pg0")
                mm(pdt, lhsT, 4096, 1024)
                ttr(1024, pdt[:], bigs[:, :1024], acc2, out_col)
            else:
                pdt = psum_d_pool.tile([128, 1024], f32, name="pg0")
                mm(pdt, lhsT, 4096, 512)
                pat = psum_d_pool.tile([128, 1024], f32, name="pg0")
                mm(pat, lhsT, 4608, 512)
                staged2 = stage_pool.tile([128, 512], f32, name="st2")
                nc.scalar.copy(staged2[:], pat[:, :512])
                ttr(512, pdt[:, :512], staged2[:], acc2, out_col)
        return

    for nt in range(NT):
        lhsT = a_sb[:, nt * 128 : (nt + 1) * 128]
        prev_acc = None

        nochain = mode.endswith("_nc")
        base = mode[:-3] if nochain else mode
        if base == "noact":
            # every 1024-group consumed directly by DVE (in1 = BIG consts)
            groups = [(0, 1024), (1024, 1024), (2048, 1024), (3072, 1024), (4096, 1024)]
        elif base in ("tail_dve", "tail"):
            # tail covers real columns 4096..4999 only (padded cols skipped)
            groups = pairs[:2] + [("tail", N - 4096)]
        elif base == "tail_first":
            # ACT-independent direct group first: DVE never waits for the
            # staged copy at tile start
            groups = [("tail", 1024)] + pairs[:2]
        else:
            groups = pairs
        rm = acc_pool.tile([128, 4], f32, name="rm") if nochain else None

        n_groups = len(groups)
        for pi, (d0, w) in enumerate(groups):
            last = pi == n_groups - 1
            if nochain:
                acc_out = rm[:, pi : pi + 1]
            elif last:
                acc_out = out_sb[:, nt : nt + 1]
            else:
                acc_out = acc_pool.tile([128, 1], f32)
            scratch = scratch_pool.tile([128, 1024], f32)

            if base == "noact":
                pool = psum_d_pool if pi % 2 == 0 else psum_a_pool
                pd = pool.tile([128, 1024], f32, name=f"pg{pi % 2}")
                mm(pd, lhsT, d0, w)
                in1 = bigs[:, :w]
            elif d0 == "tail":
                pd = psum_d_pool.tile([128, 1024], f32, name="pg0")
                mm(pd, lhsT, 4096, w)
                in1 = bigs[:, :w]
            else:
                a0 = d0 + w  # act group sits right after the dve group
                pd = psum_d_pool.tile([128, 1024], f32, name="pg0")
                pa = psum_a_pool.tile([128, 1024], f32, name="pg1")
                for i in range(w // 512):
                    nc.tensor.matmul(
                        pd[:, 512 * i : 512 * (i + 1)], lhsT,
                        b_sb[:, d0 + 512 * i : d0 + 512 * (i + 1)],
                        start=True, stop=True,
                    )
                    nc.tensor.matmul(
                        pa[:, 512 * i : 512 * (i + 1)], lhsT,
                        b_sb[:, a0 + 512 * i : a0 + 512 * (i + 1)],
                        start=True, stop=True,
                    )
                staged = stage_pool.tile([128, 1024], f32)
                nc.scalar.copy(staged[:, :w], pa[:, :w])
                in1 = staged[:, :w]

            nc.vector._custom_dve(
                ttmin,
                out=scratch[:, :w],
                in0=pd[:, :w],
                in1=in1,
                s0=BIG if (nochain or prev_acc is None) else prev_acc,
                accum_out=acc_out,
            )
            prev_acc = acc_out
        if nochain:
            nc.vector.tensor_reduce(
                out=out_sb[:, nt : nt + 1],
                in_=rm[:, :n_groups],
                axis=mybir.AxisListType.X,
                op=mybir.AluOpType.min,
            )


def _get_program():
    global _compiled
    if _compiled is None:
        _compiled = _build_program()
    return _compiled


def _make_core_inputs(x, y, mode="tail_dve"):
    """x: query points [N,3] f32, y: database points [N,3] f32.
    Returns (a_aug [K,NP], b_aug [K,MP]) so that (a_aug.T @ b_aug)[n,m] = d2.
    decomp mode: b_aug columns [0:2560) hold 0.5*(aug(y_2j)+aug(y_2j+1)),
    [2560:5120) hold 0.5*(aug(y_2j)-aug(y_2j+1))."""
    a = np.zeros((K, NP), dtype=np.float32)
    a[0:3, :N] = -2.0 * x.T
    a[3, :N] = (x * x).sum(axis=1)
    a[4, :N] = 1.0
    aug = np.zeros((K, N), dtype=np.float32)
    aug[0:3] = y.T
    aug[3] = 1.0
    aug[4] = (y * y).sum(axis=1)
    b = np.zeros((K, MP), dtype=np.float32)
    if mode == "decomp":
        ya, yb = aug[:, 0::2], aug[:, 1::2]  # [K, 2500] each
        b[:, :REAL_PAIRS] = 0.5 * (ya + yb)
        b[:, REAL_PAIRS:PAIRS_M] = 0.0
        b[4, REAL_PAIRS:PAIRS_M] = BIG
        b[:, PAIRS_M : PAIRS_M + REAL_PAIRS] = 0.5 * (ya - yb)
    else:
        b[:, :N] = aug
        b[4, N:] = BIG  # padded columns never win the min
    return a, b


def _run(pred_samples, gt_samples, trace=False):
    from concourse.bass_utils import run_bass_kernel_spmd

    nc = _get_program()
    pred = np.asarray(pred_samples, dtype=np.float32)
    gt = np.asarray(gt_samples, dtype=np.float32)
    in_maps = []
    for c in range(8):
        bidx = c % 4
        if c < 4:
            a, bb = _make_core_inputs(pred[bidx], gt[bidx], mode=SHIP_INPUT_MODE)
        else:
            a, bb = _make_core_inputs(gt[bidx], pred[bidx], mode=SHIP_INPUT_MODE)
        in_maps.append({"a_aug": a, "b_aug": bb})
    res = run_bass_kernel_spmd(nc, in_maps, list(range(8)), trace=trace)
    return res


def _gather(res):
    total = 0.0
    for c in range(8):
        mv = res.results[c]["minvals"]  # [128, NT]
        mins = mv.transpose(1, 0).reshape(-1)[:N].astype(np.float64)
        mins = np.maximum(mins, 0.0)
        total += mins.mean()
    return np.float32(total / 4.0)


def kernel(pred_samples, gt_samples):
    res = _run(pred_samples, gt_samples)
    return _gather(res)
